# revision 17
# baseline (speedup 1.0000x reference)
"""KNN-classifier kernel for Trainium2 (8 NeuronCores, SPMD).

Strategy:
  - Shard train_features row-wise across 8 cores (12500 rows each),
    single launch per core; q resident in SBUF, t-shard streamed in 5
    double-buffered chunks.
  - sim = features_rank @ shard.T in ONE fp8 e4m3 pass with
    perf_mode=DoubleRow (256-deep contraction per matmul, fp32 PSUM
    accumulation), ti-outer/k2-inner so PSUM banks recycle smoothly.
  - Extraction (the old DVE bottleneck, 493us of MAX8/FIND_INDEX8
    scans): instead of top-8 per 500-col tile, the 5 tiles of each
    (chunk g, b-block) group are max-FOLDED into one 500-wide array
    (3 cheap 2x-mode tensor_max on SBUF bf16 + 1 direct-from-PSUM
    tensor_max), then ONE top-8 scan per group. 1/5th the scan work;
    DVE ~240us, ACT ~260us, both under the PE's 371us.
  - Which of the 5 tiles a folded max came from is NOT resolved on
    device: the host expands each candidate into its 5 possible "twin"
    columns (value = folded max for all 5), merges 8x40x5 = 1600
    per-row candidates, takes top-200 by approx value, exactly
    rescores (fp32 dot) the top-96 -- the true top handful are inside
    by a wide margin, and at T=0.07 the softmax weights of everything
    outside the true top few underflow to exactly 0 -- then softmax +
    weighted class histograms exactly mirroring the reference math.
"""

import sys

sys.path.insert(0, "/opt/trn_rl_repo")

import numpy as np

B = 2048
D = 1024
NTRAIN = 100000
NCORES = 8
NLOC = NTRAIN // NCORES    # 12500
TS = 500
NT = NLOC // TS            # 25
GT = 5                     # tiles per group == tiles per chunk step
NG = NT // GT              # 5 chunks
KC = D // 128              # 8 x 128 contraction chunks (4 DoubleRow pairs)
KP = KC // 2               # 4 pairs
BT = B // 128
CPT = NG * 8               # 40 candidate slots per row per core
TPAD = GT * TS + 12        # 2512: k-dim stride %16==0 for DoubleRow APs
SB_BUFS = 8                # s16 slots; 4 consumed per block -> reuse @ 2 blocks
ACT_TILES = 4              # tiles 0..3 ACT-copied to SBUF; tile 4 folded from PSUM
MAXK = 200
TEMP = 0.07
NB_KNN = (10, 20, 100, 200)
NUM_CLASSES = 1000
RESCORE_POOL = 96

_CACHE = {}


def _build():
    from concourse import bass, tile, mybir

    if not getattr(tile.TileContext, "_drain_split_patched", False):
        from concourse.vector_clock import ScopedClock

        def _split_drain(self, tick_clock, wait_clock):
            drain_inst = self.nc.sync.drain()
            wait_clock.add_sem_waits(
                drain_inst.ins, ScopedClock({None: tick_clock.global_clock})
            )
            si = drain_inst.ins.sync_info
            if si is not None and si.on_wait and len(si.on_wait) > 1:
                waits = list(si.on_wait)
                try:
                    si.on_wait[:] = waits[:1]
                except Exception:
                    drain_inst.ins.sync_info = mybir.SyncInfo(
                        on_wait=waits[:1], on_update=list(si.on_update))
                for wt in waits[1:]:
                    d2 = self.nc.sync.drain()
                    s2 = d2.ins.sync_info
                    if s2 is None:
                        d2.ins.sync_info = mybir.SyncInfo(
                            on_wait=[wt], on_update=[])
                    else:
                        try:
                            s2.on_wait[:] = [wt]
                        except Exception:
                            d2.ins.sync_info = mybir.SyncInfo(
                                on_wait=[wt], on_update=list(s2.on_update))
            self.nc.all_engine_barrier()
            popped = self.nc._tile_sem_poison_stack.pop()
            assert popped is self._sem_poison
            self.nc.clear_and_free_semaphores(
                list(self.sems.allocated().values()))
            self.nc.all_engine_barrier()

        tile.TileContext._drain_and_barrier = _split_drain
        tile.TileContext._drain_split_patched = True

    F8 = mybir.dt.float8e4
    BF16 = mybir.dt.bfloat16
    F32 = mybir.dt.float32
    U32 = mybir.dt.uint32
    DR = mybir.MatmulPerfMode.DoubleRow

    nc = bass.Bass()
    qT = nc.declare_dram_parameter("qT", [BT, D, 128], F8, isOutput=False)
    tT = nc.declare_dram_parameter("tT", [D, NLOC], F8, isOutput=False)
    # outputs are per-chunk p-major [g, partition, b*8+o]: each chunk's
    # candidates stream out mid-kernel as one contiguous 128-descriptor
    # DMA; the host un-permutes.
    out_v = nc.declare_dram_parameter("out_v", [NG, 128, BT * 8], BF16,
                                      isOutput=True)
    out_i = nc.declare_dram_parameter("out_i", [NG, 128, BT * 8], U32,
                                      isOutput=True)

    qT4 = qT.rearrange("t (k p) b -> p t k b", p=128)
    tT3 = tT.rearrange("(k p) n -> p k n", p=128)

    with tile.TileContext(nc) as tc:
        with (
            tc.tile_pool(name="qpool", bufs=1) as qpool,
            tc.tile_pool(name="wpool", bufs=1) as wpool,
            tc.tile_pool(name="tpool", bufs=2) as tpool,
            tc.tile_pool(name="bpool", bufs=SB_BUFS) as bpool,
            tc.tile_pool(name="apool", bufs=4) as apool,
            tc.tile_pool(name="spool", bufs=1) as spool,
            tc.tile_pool(name="ppool", bufs=8, space="PSUM") as ppool,
        ):
            # HAM warm-up: ~9 matmuls on a zeroed dummy tile keep the PE
            # busy through its 3.4us activity window while the input DMAs
            # stream, so the real MM stream starts un-throttled (2.4GHz).
            w8 = wpool.tile([128, 2, 512], F8, name="warm8")
            nc.vector.memset(w8[:], 0)
            pwarm = ppool.tile([128, TS], F32, tag="ps", name="ps_warm")
            for _ in range(9):
                nc.tensor.matmul(
                    out=pwarm[:], lhsT=w8[:, :, 0:128],
                    rhs=w8[:, :, 0:TS],
                    perf_mode=DR, start=True, stop=True)

            # q8 split per b-block and t8 chunk 0 split per ti, all on
            # the GpSimd queue ordered so the first real MM only needs
            # 128KB (q8 b0) + 515KB (t8 ti0), not the full 4.5MB.
            q8 = qpool.tile([128, BT, KC, 128], F8)
            nc.gpsimd.dma_start(out=q8[:, 0], in_=qT4[:, 0])
            vals16 = spool.tile([128, NG, BT, 8], BF16, name="vals16")
            idx32 = spool.tile([128, NG, BT, 8], U32, name="idx32")

            for g in range(NG):
                t8 = tpool.tile([128, KC, TPAD], F8, tag="t8")
                if g == 0:
                    for ti in range(GT):
                        nc.gpsimd.dma_start(
                            out=t8[:, :, ti * TS:(ti + 1) * TS],
                            in_=tT3[:, :, ti * TS:(ti + 1) * TS])
                    for b in range(1, BT):
                        nc.gpsimd.dma_start(out=q8[:, b], in_=qT4[:, b])
                else:
                    nc.gpsimd.dma_start(
                        out=t8[:, :, :GT * TS],
                        in_=tT3[:, :, g * GT * TS:(g + 1) * GT * TS])
                for b in range(BT):
                    # PE-queue wait-absorbers for the DMA completions,
                    # placed just-in-time so they never stall the queue:
                    # one per q8 b-slice, one per t8 (sub-)chunk.
                    nc.tensor.ldweights(weights=q8[:, b, 0, 0:128])
                    if g > 0 and b == 0:
                        nc.tensor.ldweights(weights=t8[:, 0, 0:128])
                    pss = [ppool.tile([128, TS], F32, tag="ps",
                                      name=f"ps_{g}_{b}_{i}")
                           for i in range(GT)]
                    # ti-outer / k2-inner: each tile's PSUM bank is
                    # complete (stop) early, so extraction starts early
                    # and banks recycle without stalling the PE.
                    for ti in range(GT):
                        if g == 0 and b == 0:
                            nc.tensor.ldweights(
                                weights=t8[:, 0, ti * TS:ti * TS + 128])
                        for k2 in range(KP):
                            nc.tensor.matmul(
                                out=pss[ti][:],
                                lhsT=q8[:, b, 2 * k2:2 * k2 + 2, :],
                                rhs=t8[:, 2 * k2:2 * k2 + 2,
                                       ti * TS:(ti + 1) * TS],
                                perf_mode=DR,
                                start=(k2 == 0), stop=(k2 == KP - 1),
                            )
                    s16 = [bpool.tile([128, TS], BF16, tag="s16",
                                      name=f"s16_{g}_{b}_{j}")
                           for j in range(ACT_TILES)]
                    for j in range(ACT_TILES):
                        nc.scalar.copy(out=s16[j][:], in_=pss[j][:])
                    # fold 5 tiles -> one 500-wide max array (ping-pong,
                    # no in-place): 3 SBUF 2x-mode folds + 1 PSUM fold.
                    acc_a = apool.tile([128, TS], BF16, tag="acc",
                                       name=f"acc_a_{g}_{b}")
                    acc_b = apool.tile([128, TS], BF16, tag="acc",
                                       name=f"acc_b_{g}_{b}")
                    nc.vector.tensor_max(
                        out=acc_a[:], in0=s16[0][:], in1=s16[1][:])
                    nc.vector.tensor_max(
                        out=acc_b[:], in0=acc_a[:], in1=s16[2][:])
                    nc.vector.tensor_max(
                        out=acc_a[:], in0=acc_b[:], in1=s16[3][:])
                    nc.vector.tensor_max(
                        out=acc_b[:], in0=acc_a[:], in1=pss[GT - 1][:])
                    nc.vector.max(out=vals16[:, g, b], in_=acc_b[:])
                    nc.vector.max_index(
                        out=idx32[:, g, b], in_max=vals16[:, g, b],
                        in_values=acc_b[:])
                # stream this chunk's candidates out while later chunks
                # compute; the post-MM tail keeps only chunk 4's DMA
                nc.gpsimd.dma_start(
                    out=out_v[g],
                    in_=vals16[:, g].rearrange("p b o -> p (b o)"))
                nc.gpsimd.dma_start(
                    out=out_i[g],
                    in_=idx32[:, g].rearrange("p b o -> p (b o)"))

    # One sync-wait max per TPB/DMA instruction on the PJRT path.
    #  - same-engine waits are dropped: every TPB engine executes its
    #    queue in order, so they are implied
    #  - DMASW waits on DMAs that also wait on PE are dropped: the PE
    #    readers being waited on already waited on that DMA (WAW covered)
    #  - any residual multi-wait TPB instruction is split: same-engine
    #    NoOps inserted immediately before it each carry one wait, so
    #    queue order preserves the AND semantics exactly
    own_sem = {"InstActivation": ("Activation",),
               "InstMax": ("Vector", "DVE"),
               "InstMaxIndex": ("Vector", "DVE"),
               "InstTensorScalarPtr": ("Vector", "DVE"),
               "InstTensorTensor": ("Vector", "DVE"),
               "InstMatmult": ("PE",), "InstLdweights": ("PE",)}
    nsplit = 0
    for blk in nc.m.functions[0].blocks:
        out_list = []
        changed = False
        for ins in blk.instructions:
            si = getattr(ins, "sync_info", None)
            if si is None or not si.on_wait or len(si.on_wait) <= 1:
                out_list.append(ins)
                continue
            waits = list(si.on_wait)
            pfx = own_sem.get(type(ins).__name__)
            if pfx is not None:
                waits = [w for w in waits
                         if not w.ant_name.startswith(pfx)]
            # DMASW lane-capacity waits on DMAs that also wait on a TPB
            # engine are transitively covered: the TPB work being waited
            # on consumed (or post-dates) every earlier input DMA, so
            # the lane's prior completions are already guaranteed.
            tpb = ("PE", "DVE", "Vector", "Activation", "Pool", "SP")
            if (type(ins).__name__ == "InstDMACopy"
                    and any(w.ant_name.startswith(tpb) for w in waits)):
                waits = [w for w in waits
                         if not w.ant_name.startswith("DMASW")]
            assert type(ins).__name__ != "InstDMACopy" or len(waits) <= 1, \
                (ins, si.on_wait)
            for w in waits[:-1]:
                nsplit += 1
                out_list.append(mybir.InstNoOp(
                    name=f"wsplit_{nsplit}",
                    engine=ins.engine,
                    bass_nofuse=True,
                    sync_info=mybir.SyncInfo(on_wait=[w], on_update=[]),
                ))
            changed = True
            try:
                si.on_wait[:] = waits[-1:]
            except Exception:
                ins.sync_info = mybir.SyncInfo(
                    on_wait=waits[-1:], on_update=list(si.on_update))
            out_list.append(ins)
        if changed:
            blk.instructions[:] = out_list
    return nc


def _run_device(q, t, trace=False):
    from concourse.bass_utils import run_bass_kernel_spmd
    import ml_dtypes

    if "nc" not in _CACHE:
        _CACHE["nc"] = _build()
    nc = _CACHE["nc"]

    q8 = q.astype(ml_dtypes.float8_e4m3)        # [B, D]
    q8 = np.ascontiguousarray(
        q8.reshape(BT, 128, D).transpose(0, 2, 1))            # [BT, D, 128]
    in_maps = []
    for c in range(NCORES):
        sh = t[c * NLOC:(c + 1) * NLOC].astype(ml_dtypes.float8_e4m3)
        in_maps.append({"qT": q8, "tT": np.ascontiguousarray(sh.T)})
    res = run_bass_kernel_spmd(nc, in_maps, core_ids=list(range(NCORES)),
                               trace=trace)
    if trace:
        _run_device.last_exec_ns = res.exec_time_ns

    # slot s of CPT=40 belongs to chunk g=s//8; device index p in [0,500)
    # is ambiguous over the group's GT=5 tiles -> expand to 5 twin
    # columns per candidate, each carrying the folded (max) value.
    g_of_slot = np.arange(CPT, dtype=np.int64) // 8          # [CPT]
    group_base = g_of_slot * (GT * TS)                        # [CPT]
    twin_off = np.arange(GT, dtype=np.int64) * TS             # [GT]
    cvs, cis = [], []
    for c in range(NCORES):
        v = res.results[c]["out_v"]                    # [NG, 128, BT*8]
        p = res.results[c]["out_i"]
        v = (v.reshape(NG, 128, BT, 8).transpose(2, 1, 0, 3)
             .reshape(B, CPT).astype(np.float32))
        p = (p.reshape(NG, 128, BT, 8).transpose(2, 1, 0, 3)
             .reshape(B, CPT).astype(np.int64))
        cols = (c * NLOC + group_base)[None, :, None] \
            + twin_off[None, None, :] + p[:, :, None]         # [B, CPT, GT]
        cvs.append(np.repeat(v, GT, axis=1))
        cis.append(cols.reshape(B, CPT * GT))
    return np.concatenate(cvs, axis=1), np.concatenate(cis, axis=1)


def kernel(features_rank, train_features, train_labels):
    q = np.ascontiguousarray(np.asarray(features_rank), dtype=np.float32)
    t = np.ascontiguousarray(np.asarray(train_features), dtype=np.float32)
    lab = np.asarray(train_labels)

    cv, ci = _run_device(q, t)

    part = np.argpartition(-cv, MAXK - 1, axis=1)[:, :MAXK]
    pv = np.take_along_axis(cv, part, axis=1)
    pi = np.take_along_axis(ci, part, axis=1)
    order = np.lexsort((pi, -pv), axis=1)
    topv = np.take_along_axis(pv, order, axis=1)
    topi = np.take_along_axis(pi, order, axis=1)

    # fp8 sims are approximate and twin values are inflated to the group
    # max: exactly rescore the top-RESCORE_POOL of every row (the true
    # top few are inside by a wide margin), re-sort, then softmax. The
    # tail keeps approx values; its true weights underflow to exactly 0
    # at T=0.07 regardless.
    p2 = RESCORE_POOL
    sub_i = topi[:, :p2]
    ex = np.empty((B, p2), np.float32)
    CH = 256
    for i in range(0, B, CH):
        ex[i:i + CH] = (t[sub_i[i:i + CH]] @ q[i:i + CH, :, None])[:, :, 0]
    o2 = np.lexsort((sub_i, -ex), axis=1)
    topv[:, :p2] = np.take_along_axis(ex, o2, axis=1)
    topi[:, :p2] = np.take_along_axis(sub_i, o2, axis=1)

    x = topv / np.float32(TEMP)
    x -= x.max(axis=1, keepdims=True)
    e = np.exp(x, dtype=np.float32)
    w = e / e.sum(axis=1, keepdims=True, dtype=np.float32)

    nl = lab[topi].astype(np.int64)
    flat_base = np.arange(B, dtype=np.int64)[:, None] * NUM_CLASSES
    probas = []
    for k in NB_KNN:
        p = np.bincount((nl[:, :k] + flat_base).ravel(),
                        weights=w[:, :k].astype(np.float64).ravel(),
                        minlength=B * NUM_CLASSES)
        probas.append(p.reshape(B, NUM_CLASSES).astype(np.float32))
    return tuple(probas)


# revision 18
# speedup vs baseline: 1.0044x; 1.0044x over previous
"""KNN-classifier kernel for Trainium2 (8 NeuronCores, SPMD).

Strategy:
  - Shard train_features row-wise across 8 cores (12500 rows each),
    single launch per core; q resident in SBUF, t-shard streamed in 5
    double-buffered chunks.
  - sim = features_rank @ shard.T in ONE fp8 e4m3 pass with
    perf_mode=DoubleRow (256-deep contraction per matmul, fp32 PSUM
    accumulation), ti-outer/k2-inner so PSUM banks recycle smoothly.
  - Extraction (the old DVE bottleneck, 493us of MAX8/FIND_INDEX8
    scans): instead of top-8 per 500-col tile, the 5 tiles of each
    (chunk g, b-block) group are max-FOLDED into one 500-wide array
    (3 cheap 2x-mode tensor_max on SBUF bf16 + 1 direct-from-PSUM
    tensor_max), then ONE top-8 scan per group. 1/5th the scan work;
    DVE ~234us, ACT ~223us, both under the PE's ~350us.
  - Which of the 5 tiles a folded max came from is NOT resolved on
    device: the host expands each candidate into its 5 possible "twin"
    columns (value = folded max for all 5), merges 8x40x5 = 1600
    per-row candidates, takes top-200 by approx value, exactly
    rescores (fp32 dot) the top-96 -- the true top handful are inside
    by a wide margin, and at T=0.07 the softmax weights of everything
    outside the true top few underflow to exactly 0 -- then softmax +
    weighted class histograms exactly mirroring the reference math.
  - Launch overheads trimmed: 9 HAM-warm-up matmuls, per-b-block q8 /
    per-ti chunk-0 DMAs so the first MM needs only ~0.6MB, per-chunk
    p-major candidate DMAs, just-in-time ldweights DMA-wait absorbers,
    and a post-Tile pass that enforces the PJRT one-sync-wait rule by
    dropping provably-implied waits and NoOp-splitting the rest.

Measured: 369-371us HW exec (NTFF, core 0; PE 93% busy at the fp8
DoubleRow roofline of ~214ns per N=500 matmul), rel_err 4.75e-4 vs the
fp32 reference (gate 2e-2). Prior-session baseline with per-tile DVE
top-8 scans: 535us (DVE-bound, 91% busy); occasional +19us outlier runs
under chip power throttle (throttle_active ~130us vs normal ~20us).
"""

import sys

sys.path.insert(0, "/opt/trn_rl_repo")

import numpy as np

B = 2048
D = 1024
NTRAIN = 100000
NCORES = 8
NLOC = NTRAIN // NCORES    # 12500
TS = 500
NT = NLOC // TS            # 25
GT = 5                     # tiles per group == tiles per chunk step
NG = NT // GT              # 5 chunks
KC = D // 128              # 8 x 128 contraction chunks (4 DoubleRow pairs)
KP = KC // 2               # 4 pairs
BT = B // 128
CPT = NG * 8               # 40 candidate slots per row per core
TPAD = GT * TS + 12        # 2512: k-dim stride %16==0 for DoubleRow APs
SB_BUFS = 8                # s16 slots; 4 consumed per block -> reuse @ 2 blocks
ACT_TILES = 4              # tiles 0..3 ACT-copied to SBUF; tile 4 folded from PSUM
MAXK = 200
TEMP = 0.07
NB_KNN = (10, 20, 100, 200)
NUM_CLASSES = 1000
RESCORE_POOL = 96

_CACHE = {}


def _build():
    from concourse import bass, tile, mybir

    if not getattr(tile.TileContext, "_drain_split_patched", False):
        from concourse.vector_clock import ScopedClock

        def _split_drain(self, tick_clock, wait_clock):
            drain_inst = self.nc.sync.drain()
            wait_clock.add_sem_waits(
                drain_inst.ins, ScopedClock({None: tick_clock.global_clock})
            )
            si = drain_inst.ins.sync_info
            if si is not None and si.on_wait and len(si.on_wait) > 1:
                waits = list(si.on_wait)
                try:
                    si.on_wait[:] = waits[:1]
                except Exception:
                    drain_inst.ins.sync_info = mybir.SyncInfo(
                        on_wait=waits[:1], on_update=list(si.on_update))
                for wt in waits[1:]:
                    d2 = self.nc.sync.drain()
                    s2 = d2.ins.sync_info
                    if s2 is None:
                        d2.ins.sync_info = mybir.SyncInfo(
                            on_wait=[wt], on_update=[])
                    else:
                        try:
                            s2.on_wait[:] = [wt]
                        except Exception:
                            d2.ins.sync_info = mybir.SyncInfo(
                                on_wait=[wt], on_update=list(s2.on_update))
            self.nc.all_engine_barrier()
            popped = self.nc._tile_sem_poison_stack.pop()
            assert popped is self._sem_poison
            self.nc.clear_and_free_semaphores(
                list(self.sems.allocated().values()))
            self.nc.all_engine_barrier()

        tile.TileContext._drain_and_barrier = _split_drain
        tile.TileContext._drain_split_patched = True

    F8 = mybir.dt.float8e4
    BF16 = mybir.dt.bfloat16
    F32 = mybir.dt.float32
    U32 = mybir.dt.uint32
    DR = mybir.MatmulPerfMode.DoubleRow

    nc = bass.Bass()
    qT = nc.declare_dram_parameter("qT", [BT, D, 128], F8, isOutput=False)
    tT = nc.declare_dram_parameter("tT", [D, NLOC], F8, isOutput=False)
    # outputs are per-chunk p-major [g, partition, b*8+o]: each chunk's
    # candidates stream out mid-kernel as one contiguous 128-descriptor
    # DMA; the host un-permutes.
    out_v = nc.declare_dram_parameter("out_v", [NG, 128, BT * 8], BF16,
                                      isOutput=True)
    out_i = nc.declare_dram_parameter("out_i", [NG, 128, BT * 8], U32,
                                      isOutput=True)

    qT4 = qT.rearrange("t (k p) b -> p t k b", p=128)
    tT3 = tT.rearrange("(k p) n -> p k n", p=128)

    with tile.TileContext(nc) as tc:
        with (
            tc.tile_pool(name="qpool", bufs=1) as qpool,
            tc.tile_pool(name="wpool", bufs=1) as wpool,
            tc.tile_pool(name="tpool", bufs=2) as tpool,
            tc.tile_pool(name="bpool", bufs=SB_BUFS) as bpool,
            tc.tile_pool(name="apool", bufs=4) as apool,
            tc.tile_pool(name="spool", bufs=1) as spool,
            tc.tile_pool(name="ppool", bufs=8, space="PSUM") as ppool,
        ):
            # HAM warm-up: ~9 matmuls on a zeroed dummy tile keep the PE
            # busy through its 3.4us activity window while the input DMAs
            # stream, so the real MM stream starts un-throttled (2.4GHz).
            w8 = wpool.tile([128, 2, 512], F8, name="warm8")
            nc.vector.memset(w8[:], 0)
            pwarm = ppool.tile([128, TS], F32, tag="ps", name="ps_warm")
            for _ in range(9):
                nc.tensor.matmul(
                    out=pwarm[:], lhsT=w8[:, :, 0:128],
                    rhs=w8[:, :, 0:TS],
                    perf_mode=DR, start=True, stop=True)

            # q8 split per b-block and t8 chunk 0 split per ti, all on
            # the GpSimd queue ordered so the first real MM only needs
            # 128KB (q8 b0) + 515KB (t8 ti0), not the full 4.5MB.
            q8 = qpool.tile([128, BT, KC, 128], F8)
            nc.gpsimd.dma_start(out=q8[:, 0], in_=qT4[:, 0])
            vals16 = spool.tile([128, NG, BT, 8], BF16, name="vals16")
            idx32 = spool.tile([128, NG, BT, 8], U32, name="idx32")

            for g in range(NG):
                t8 = tpool.tile([128, KC, TPAD], F8, tag="t8")
                if g == 0:
                    for ti in range(GT):
                        nc.gpsimd.dma_start(
                            out=t8[:, :, ti * TS:(ti + 1) * TS],
                            in_=tT3[:, :, ti * TS:(ti + 1) * TS])
                    for b in range(1, BT):
                        nc.gpsimd.dma_start(out=q8[:, b], in_=qT4[:, b])
                else:
                    nc.gpsimd.dma_start(
                        out=t8[:, :, :GT * TS],
                        in_=tT3[:, :, g * GT * TS:(g + 1) * GT * TS])
                for b in range(BT):
                    # PE-queue wait-absorbers for the DMA completions,
                    # placed just-in-time so they never stall the queue:
                    # one per q8 b-slice, one per t8 (sub-)chunk.
                    nc.tensor.ldweights(weights=q8[:, b, 0, 0:128])
                    if g > 0 and b == 0:
                        nc.tensor.ldweights(weights=t8[:, 0, 0:128])
                    pss = [ppool.tile([128, TS], F32, tag="ps",
                                      name=f"ps_{g}_{b}_{i}")
                           for i in range(GT)]
                    # ti-outer / k2-inner: each tile's PSUM bank is
                    # complete (stop) early, so extraction starts early
                    # and banks recycle without stalling the PE.
                    for ti in range(GT):
                        if g == 0 and b == 0:
                            nc.tensor.ldweights(
                                weights=t8[:, 0, ti * TS:ti * TS + 128])
                        for k2 in range(KP):
                            nc.tensor.matmul(
                                out=pss[ti][:],
                                lhsT=q8[:, b, 2 * k2:2 * k2 + 2, :],
                                rhs=t8[:, 2 * k2:2 * k2 + 2,
                                       ti * TS:(ti + 1) * TS],
                                perf_mode=DR,
                                start=(k2 == 0), stop=(k2 == KP - 1),
                            )
                    s16 = [bpool.tile([128, TS], BF16, tag="s16",
                                      name=f"s16_{g}_{b}_{j}")
                           for j in range(ACT_TILES)]
                    for j in range(ACT_TILES):
                        nc.scalar.copy(out=s16[j][:], in_=pss[j][:])
                    # fold 5 tiles -> one 500-wide max array (ping-pong,
                    # no in-place): 3 SBUF 2x-mode folds + 1 PSUM fold.
                    acc_a = apool.tile([128, TS], BF16, tag="acc",
                                       name=f"acc_a_{g}_{b}")
                    acc_b = apool.tile([128, TS], BF16, tag="acc",
                                       name=f"acc_b_{g}_{b}")
                    nc.vector.tensor_max(
                        out=acc_a[:], in0=s16[0][:], in1=s16[1][:])
                    nc.vector.tensor_max(
                        out=acc_b[:], in0=acc_a[:], in1=s16[2][:])
                    nc.vector.tensor_max(
                        out=acc_a[:], in0=acc_b[:], in1=s16[3][:])
                    nc.vector.tensor_max(
                        out=acc_b[:], in0=acc_a[:], in1=pss[GT - 1][:])
                    nc.vector.max(out=vals16[:, g, b], in_=acc_b[:])
                    nc.vector.max_index(
                        out=idx32[:, g, b], in_max=vals16[:, g, b],
                        in_values=acc_b[:])
                # stream this chunk's candidates out while later chunks
                # compute; the post-MM tail keeps only chunk 4's DMA
                nc.gpsimd.dma_start(
                    out=out_v[g],
                    in_=vals16[:, g].rearrange("p b o -> p (b o)"))
                nc.gpsimd.dma_start(
                    out=out_i[g],
                    in_=idx32[:, g].rearrange("p b o -> p (b o)"))

    # One sync-wait max per TPB/DMA instruction on the PJRT path.
    #  - same-engine waits are dropped: every TPB engine executes its
    #    queue in order, so they are implied
    #  - DMASW waits on DMAs that also wait on PE are dropped: the PE
    #    readers being waited on already waited on that DMA (WAW covered)
    #  - any residual multi-wait TPB instruction is split: same-engine
    #    NoOps inserted immediately before it each carry one wait, so
    #    queue order preserves the AND semantics exactly
    own_sem = {"InstActivation": ("Activation",),
               "InstMax": ("Vector", "DVE"),
               "InstMaxIndex": ("Vector", "DVE"),
               "InstTensorScalarPtr": ("Vector", "DVE"),
               "InstTensorTensor": ("Vector", "DVE"),
               "InstMatmult": ("PE",), "InstLdweights": ("PE",)}
    nsplit = 0
    for blk in nc.m.functions[0].blocks:
        out_list = []
        changed = False
        for ins in blk.instructions:
            si = getattr(ins, "sync_info", None)
            if si is None or not si.on_wait or len(si.on_wait) <= 1:
                out_list.append(ins)
                continue
            waits = list(si.on_wait)
            pfx = own_sem.get(type(ins).__name__)
            if pfx is not None:
                waits = [w for w in waits
                         if not w.ant_name.startswith(pfx)]
            # DMASW lane-capacity waits on DMAs that also wait on a TPB
            # engine are transitively covered: the TPB work being waited
            # on consumed (or post-dates) every earlier input DMA, so
            # the lane's prior completions are already guaranteed.
            tpb = ("PE", "DVE", "Vector", "Activation", "Pool", "SP")
            if (type(ins).__name__ == "InstDMACopy"
                    and any(w.ant_name.startswith(tpb) for w in waits)):
                waits = [w for w in waits
                         if not w.ant_name.startswith("DMASW")]
            assert type(ins).__name__ != "InstDMACopy" or len(waits) <= 1, \
                (ins, si.on_wait)
            for w in waits[:-1]:
                nsplit += 1
                out_list.append(mybir.InstNoOp(
                    name=f"wsplit_{nsplit}",
                    engine=ins.engine,
                    bass_nofuse=True,
                    sync_info=mybir.SyncInfo(on_wait=[w], on_update=[]),
                ))
            changed = True
            try:
                si.on_wait[:] = waits[-1:]
            except Exception:
                ins.sync_info = mybir.SyncInfo(
                    on_wait=waits[-1:], on_update=list(si.on_update))
            out_list.append(ins)
        if changed:
            blk.instructions[:] = out_list
    return nc


def _run_device(q, t, trace=False):
    from concourse.bass_utils import run_bass_kernel_spmd
    import ml_dtypes

    if "nc" not in _CACHE:
        _CACHE["nc"] = _build()
    nc = _CACHE["nc"]

    q8 = q.astype(ml_dtypes.float8_e4m3)        # [B, D]
    q8 = np.ascontiguousarray(
        q8.reshape(BT, 128, D).transpose(0, 2, 1))            # [BT, D, 128]
    in_maps = []
    for c in range(NCORES):
        sh = t[c * NLOC:(c + 1) * NLOC].astype(ml_dtypes.float8_e4m3)
        in_maps.append({"qT": q8, "tT": np.ascontiguousarray(sh.T)})
    res = run_bass_kernel_spmd(nc, in_maps, core_ids=list(range(NCORES)),
                               trace=trace)
    if trace:
        _run_device.last_exec_ns = res.exec_time_ns

    # slot s of CPT=40 belongs to chunk g=s//8; device index p in [0,500)
    # is ambiguous over the group's GT=5 tiles -> expand to 5 twin
    # columns per candidate, each carrying the folded (max) value.
    g_of_slot = np.arange(CPT, dtype=np.int64) // 8          # [CPT]
    group_base = g_of_slot * (GT * TS)                        # [CPT]
    twin_off = np.arange(GT, dtype=np.int64) * TS             # [GT]
    cvs, cis = [], []
    for c in range(NCORES):
        v = res.results[c]["out_v"]                    # [NG, 128, BT*8]
        p = res.results[c]["out_i"]
        v = (v.reshape(NG, 128, BT, 8).transpose(2, 1, 0, 3)
             .reshape(B, CPT).astype(np.float32))
        p = (p.reshape(NG, 128, BT, 8).transpose(2, 1, 0, 3)
             .reshape(B, CPT).astype(np.int64))
        cols = (c * NLOC + group_base)[None, :, None] \
            + twin_off[None, None, :] + p[:, :, None]         # [B, CPT, GT]
        cvs.append(np.repeat(v, GT, axis=1))
        cis.append(cols.reshape(B, CPT * GT))
    return np.concatenate(cvs, axis=1), np.concatenate(cis, axis=1)


def kernel(features_rank, train_features, train_labels):
    q = np.ascontiguousarray(np.asarray(features_rank), dtype=np.float32)
    t = np.ascontiguousarray(np.asarray(train_features), dtype=np.float32)
    lab = np.asarray(train_labels)

    cv, ci = _run_device(q, t)

    part = np.argpartition(-cv, MAXK - 1, axis=1)[:, :MAXK]
    pv = np.take_along_axis(cv, part, axis=1)
    pi = np.take_along_axis(ci, part, axis=1)
    order = np.lexsort((pi, -pv), axis=1)
    topv = np.take_along_axis(pv, order, axis=1)
    topi = np.take_along_axis(pi, order, axis=1)

    # fp8 sims are approximate and twin values are inflated to the group
    # max: exactly rescore the top-RESCORE_POOL of every row (the true
    # top few are inside by a wide margin), re-sort, then softmax. The
    # tail keeps approx values; its true weights underflow to exactly 0
    # at T=0.07 regardless.
    p2 = RESCORE_POOL
    sub_i = topi[:, :p2]
    ex = np.empty((B, p2), np.float32)
    CH = 256
    for i in range(0, B, CH):
        ex[i:i + CH] = (t[sub_i[i:i + CH]] @ q[i:i + CH, :, None])[:, :, 0]
    o2 = np.lexsort((sub_i, -ex), axis=1)
    topv[:, :p2] = np.take_along_axis(ex, o2, axis=1)
    topi[:, :p2] = np.take_along_axis(sub_i, o2, axis=1)

    x = topv / np.float32(TEMP)
    x -= x.max(axis=1, keepdims=True)
    e = np.exp(x, dtype=np.float32)
    w = e / e.sum(axis=1, keepdims=True, dtype=np.float32)

    nl = lab[topi].astype(np.int64)
    flat_base = np.arange(B, dtype=np.int64)[:, None] * NUM_CLASSES
    probas = []
    for k in NB_KNN:
        p = np.bincount((nl[:, :k] + flat_base).ravel(),
                        weights=w[:, :k].astype(np.float64).ravel(),
                        minlength=B * NUM_CLASSES)
        probas.append(p.reshape(B, NUM_CLASSES).astype(np.float32))
    return tuple(probas)


# revision 19
# speedup vs baseline: 1.0044x; 1.0000x over previous
"""KNN-classifier kernel for Trainium2 (8 NeuronCores, SPMD).

Strategy:
  - Shard train_features row-wise across 8 cores (12500 rows each),
    single launch per core; q resident in SBUF, t-shard streamed in 5
    double-buffered chunks.
  - sim = features_rank @ shard.T in ONE fp8 e4m3 pass with
    perf_mode=DoubleRow (256-deep contraction per matmul, fp32 PSUM
    accumulation), ti-outer/k2-inner so PSUM banks recycle smoothly.
  - Extraction (the old DVE bottleneck, 493us of MAX8/FIND_INDEX8
    scans): instead of top-8 per 500-col tile, the 5 tiles of each
    (chunk g, b-block) group are max-FOLDED into one 500-wide array
    (3 cheap 2x-mode tensor_max on SBUF bf16 + 1 direct-from-PSUM
    tensor_max), then ONE top-8 scan per group. 1/5th the scan work;
    DVE ~234us, ACT ~223us, both under the PE's ~350us.
  - Which of the 5 tiles a folded max came from is NOT resolved on
    device: the host expands each candidate into its 5 possible "twin"
    columns (value = folded max for all 5), merges 8x40x5 = 1600
    per-row candidates, takes top-200 by approx value, exactly
    rescores (fp32 dot) the top-96 -- the true top handful are inside
    by a wide margin, and at T=0.07 the softmax weights of everything
    outside the true top few underflow to exactly 0 -- then softmax +
    weighted class histograms exactly mirroring the reference math.
  - Launch overheads trimmed: 9 HAM-warm-up matmuls, per-b-block q8 /
    per-ti chunk-0 DMAs so the first MM needs only ~0.6MB, per-chunk
    p-major candidate DMAs, just-in-time ldweights DMA-wait absorbers,
    and a post-Tile pass that enforces the PJRT one-sync-wait rule by
    dropping provably-implied waits and NoOp-splitting the rest.

Measured: 369-371us HW exec (NTFF, core 0; PE 93% busy at the fp8
DoubleRow roofline of ~214ns per N=500 matmul), rel_err 4.75e-4 vs the
fp32 reference (gate 2e-2). Prior-session baseline with per-tile DVE
top-8 scans: 535us (DVE-bound, 91% busy); occasional +19us outlier runs
under chip power throttle (throttle_active ~130us vs normal ~20us).
"""

import sys

sys.path.insert(0, "/opt/trn_rl_repo")

import numpy as np

B = 2048
D = 1024
NTRAIN = 100000
NCORES = 8
NLOC = NTRAIN // NCORES    # 12500
TS = 500
NT = NLOC // TS            # 25
GT = 5                     # tiles per group == tiles per chunk step
NG = NT // GT              # 5 chunks
KC = D // 128              # 8 x 128 contraction chunks (4 DoubleRow pairs)
KP = KC // 2               # 4 pairs
BT = B // 128
CPT = NG * 8               # 40 candidate slots per row per core
TPAD = GT * TS + 12        # 2512: k-dim stride %16==0 for DoubleRow APs
SB_BUFS = 8                # s16 slots; 4 consumed per block -> reuse @ 2 blocks
ACT_TILES = 4              # tiles 0..3 ACT-copied to SBUF; tile 4 folded from PSUM
MAXK = 200
TEMP = 0.07
NB_KNN = (10, 20, 100, 200)
NUM_CLASSES = 1000
RESCORE_POOL = 96

_CACHE = {}


def _build():
    from concourse import bass, tile, mybir

    if not getattr(tile.TileContext, "_drain_split_patched", False):
        from concourse.vector_clock import ScopedClock

        def _split_drain(self, tick_clock, wait_clock):
            drain_inst = self.nc.sync.drain()
            wait_clock.add_sem_waits(
                drain_inst.ins, ScopedClock({None: tick_clock.global_clock})
            )
            si = drain_inst.ins.sync_info
            if si is not None and si.on_wait and len(si.on_wait) > 1:
                waits = list(si.on_wait)
                try:
                    si.on_wait[:] = waits[:1]
                except Exception:
                    drain_inst.ins.sync_info = mybir.SyncInfo(
                        on_wait=waits[:1], on_update=list(si.on_update))
                for wt in waits[1:]:
                    d2 = self.nc.sync.drain()
                    s2 = d2.ins.sync_info
                    if s2 is None:
                        d2.ins.sync_info = mybir.SyncInfo(
                            on_wait=[wt], on_update=[])
                    else:
                        try:
                            s2.on_wait[:] = [wt]
                        except Exception:
                            d2.ins.sync_info = mybir.SyncInfo(
                                on_wait=[wt], on_update=list(s2.on_update))
            self.nc.all_engine_barrier()
            popped = self.nc._tile_sem_poison_stack.pop()
            assert popped is self._sem_poison
            self.nc.clear_and_free_semaphores(
                list(self.sems.allocated().values()))
            self.nc.all_engine_barrier()

        tile.TileContext._drain_and_barrier = _split_drain
        tile.TileContext._drain_split_patched = True

    F8 = mybir.dt.float8e4
    BF16 = mybir.dt.bfloat16
    F32 = mybir.dt.float32
    U32 = mybir.dt.uint32
    DR = mybir.MatmulPerfMode.DoubleRow

    nc = bass.Bass()
    qT = nc.declare_dram_parameter("qT", [BT, D, 128], F8, isOutput=False)
    tT = nc.declare_dram_parameter("tT", [D, NLOC], F8, isOutput=False)
    # outputs are per-chunk p-major [g, partition, b*8+o]: each chunk's
    # candidates stream out mid-kernel as one contiguous 128-descriptor
    # DMA; the host un-permutes.
    out_v = nc.declare_dram_parameter("out_v", [NG, 128, BT * 8], BF16,
                                      isOutput=True)
    out_i = nc.declare_dram_parameter("out_i", [NG, 128, BT * 8], U32,
                                      isOutput=True)

    qT4 = qT.rearrange("t (k p) b -> p t k b", p=128)
    tT3 = tT.rearrange("(k p) n -> p k n", p=128)

    with tile.TileContext(nc) as tc:
        with (
            tc.tile_pool(name="qpool", bufs=1) as qpool,
            tc.tile_pool(name="wpool", bufs=1) as wpool,
            tc.tile_pool(name="tpool", bufs=2) as tpool,
            tc.tile_pool(name="bpool", bufs=SB_BUFS) as bpool,
            tc.tile_pool(name="apool", bufs=4) as apool,
            tc.tile_pool(name="spool", bufs=1) as spool,
            tc.tile_pool(name="ppool", bufs=8, space="PSUM") as ppool,
        ):
            # HAM warm-up: dummy matmuls on a zeroed tile keep the PE
            # busy from the ~3us preamble end until the first input data
            # lands (~11us), so the real MM stream starts un-throttled
            # (2.4GHz) with no >3.4us idle window in between.
            w8 = wpool.tile([128, 2, 512], F8, name="warm8")
            nc.vector.memset(w8[:], 0)
            pwarm = ppool.tile([128, TS], F32, tag="ps", name="ps_warm")
            for _ in range(24):
                nc.tensor.matmul(
                    out=pwarm[:], lhsT=w8[:, :, 0:128],
                    rhs=w8[:, :, 0:TS],
                    perf_mode=DR, start=True, stop=True)

            # q8 split per b-block and t8 chunk 0 split per ti, all on
            # the GpSimd queue ordered so the first real MM only needs
            # 128KB (q8 b0) + 515KB (t8 ti0), not the full 4.5MB.
            q8 = qpool.tile([128, BT, KC, 128], F8)
            vals16 = spool.tile([128, NG, BT, 8], BF16, name="vals16")
            idx32 = spool.tile([128, NG, BT, 8], U32, name="idx32")

            for g in range(NG):
                t8 = tpool.tile([128, KC, TPAD], F8, tag="t8")
                if g == 0:
                    # ti0 first (largest critical transfer), then q8 b0,
                    # then the rest
                    nc.gpsimd.dma_start(
                        out=t8[:, :, 0:TS], in_=tT3[:, :, 0:TS])
                    nc.gpsimd.dma_start(out=q8[:, 0], in_=qT4[:, 0])
                    for ti in range(1, GT):
                        nc.gpsimd.dma_start(
                            out=t8[:, :, ti * TS:(ti + 1) * TS],
                            in_=tT3[:, :, ti * TS:(ti + 1) * TS])
                    for b in range(1, BT):
                        nc.gpsimd.dma_start(out=q8[:, b], in_=qT4[:, b])
                else:
                    nc.gpsimd.dma_start(
                        out=t8[:, :, :GT * TS],
                        in_=tT3[:, :, g * GT * TS:(g + 1) * GT * TS])
                for b in range(BT):
                    # PE-queue wait-absorbers for the DMA completions,
                    # placed just-in-time so they never stall the queue:
                    # one per q8 b-slice, one per t8 (sub-)chunk.
                    nc.tensor.ldweights(weights=q8[:, b, 0, 0:128])
                    if g > 0 and b == 0:
                        nc.tensor.ldweights(weights=t8[:, 0, 0:128])
                    pss = [ppool.tile([128, TS], F32, tag="ps",
                                      name=f"ps_{g}_{b}_{i}")
                           for i in range(GT)]
                    # ti-outer / k2-inner: each tile's PSUM bank is
                    # complete (stop) early, so extraction starts early
                    # and banks recycle without stalling the PE.
                    for ti in range(GT):
                        if g == 0 and b == 0:
                            nc.tensor.ldweights(
                                weights=t8[:, 0, ti * TS:ti * TS + 128])
                        for k2 in range(KP):
                            nc.tensor.matmul(
                                out=pss[ti][:],
                                lhsT=q8[:, b, 2 * k2:2 * k2 + 2, :],
                                rhs=t8[:, 2 * k2:2 * k2 + 2,
                                       ti * TS:(ti + 1) * TS],
                                perf_mode=DR,
                                start=(k2 == 0), stop=(k2 == KP - 1),
                            )
                    s16 = [bpool.tile([128, TS], BF16, tag="s16",
                                      name=f"s16_{g}_{b}_{j}")
                           for j in range(ACT_TILES)]
                    for j in range(ACT_TILES):
                        nc.scalar.copy(out=s16[j][:], in_=pss[j][:])
                    # fold 5 tiles -> one 500-wide max array (ping-pong,
                    # no in-place): 3 SBUF 2x-mode folds + 1 PSUM fold.
                    acc_a = apool.tile([128, TS], BF16, tag="acc",
                                       name=f"acc_a_{g}_{b}")
                    acc_b = apool.tile([128, TS], BF16, tag="acc",
                                       name=f"acc_b_{g}_{b}")
                    nc.vector.tensor_max(
                        out=acc_a[:], in0=s16[0][:], in1=s16[1][:])
                    nc.vector.tensor_max(
                        out=acc_b[:], in0=acc_a[:], in1=s16[2][:])
                    nc.vector.tensor_max(
                        out=acc_a[:], in0=acc_b[:], in1=s16[3][:])
                    nc.vector.tensor_max(
                        out=acc_b[:], in0=acc_a[:], in1=pss[GT - 1][:])
                    nc.vector.max(out=vals16[:, g, b], in_=acc_b[:])
                    nc.vector.max_index(
                        out=idx32[:, g, b], in_max=vals16[:, g, b],
                        in_values=acc_b[:])
                # stream this chunk's candidates out while later chunks
                # compute; the post-MM tail keeps only chunk 4's DMA
                nc.gpsimd.dma_start(
                    out=out_v[g],
                    in_=vals16[:, g].rearrange("p b o -> p (b o)"))
                nc.gpsimd.dma_start(
                    out=out_i[g],
                    in_=idx32[:, g].rearrange("p b o -> p (b o)"))

    # One sync-wait max per TPB/DMA instruction on the PJRT path.
    #  - same-engine waits are dropped: every TPB engine executes its
    #    queue in order, so they are implied
    #  - DMASW waits on DMAs that also wait on PE are dropped: the PE
    #    readers being waited on already waited on that DMA (WAW covered)
    #  - any residual multi-wait TPB instruction is split: same-engine
    #    NoOps inserted immediately before it each carry one wait, so
    #    queue order preserves the AND semantics exactly
    own_sem = {"InstActivation": ("Activation",),
               "InstMax": ("Vector", "DVE"),
               "InstMaxIndex": ("Vector", "DVE"),
               "InstTensorScalarPtr": ("Vector", "DVE"),
               "InstTensorTensor": ("Vector", "DVE"),
               "InstMatmult": ("PE",), "InstLdweights": ("PE",)}
    nsplit = 0
    for blk in nc.m.functions[0].blocks:
        out_list = []
        changed = False
        for ins in blk.instructions:
            si = getattr(ins, "sync_info", None)
            if si is None or not si.on_wait or len(si.on_wait) <= 1:
                out_list.append(ins)
                continue
            waits = list(si.on_wait)
            pfx = own_sem.get(type(ins).__name__)
            if pfx is not None:
                waits = [w for w in waits
                         if not w.ant_name.startswith(pfx)]
            # DMASW lane-capacity waits on DMAs that also wait on a TPB
            # engine are transitively covered: the TPB work being waited
            # on consumed (or post-dates) every earlier input DMA, so
            # the lane's prior completions are already guaranteed.
            tpb = ("PE", "DVE", "Vector", "Activation", "Pool", "SP")
            if (type(ins).__name__ == "InstDMACopy"
                    and any(w.ant_name.startswith(tpb) for w in waits)):
                waits = [w for w in waits
                         if not w.ant_name.startswith("DMASW")]
            assert type(ins).__name__ != "InstDMACopy" or len(waits) <= 1, \
                (ins, si.on_wait)
            for w in waits[:-1]:
                nsplit += 1
                out_list.append(mybir.InstNoOp(
                    name=f"wsplit_{nsplit}",
                    engine=ins.engine,
                    bass_nofuse=True,
                    sync_info=mybir.SyncInfo(on_wait=[w], on_update=[]),
                ))
            changed = True
            try:
                si.on_wait[:] = waits[-1:]
            except Exception:
                ins.sync_info = mybir.SyncInfo(
                    on_wait=waits[-1:], on_update=list(si.on_update))
            out_list.append(ins)
        if changed:
            blk.instructions[:] = out_list
    return nc


def _run_device(q, t, trace=False):
    from concourse.bass_utils import run_bass_kernel_spmd
    import ml_dtypes

    if "nc" not in _CACHE:
        _CACHE["nc"] = _build()
    nc = _CACHE["nc"]

    q8 = q.astype(ml_dtypes.float8_e4m3)        # [B, D]
    q8 = np.ascontiguousarray(
        q8.reshape(BT, 128, D).transpose(0, 2, 1))            # [BT, D, 128]
    in_maps = []
    for c in range(NCORES):
        sh = t[c * NLOC:(c + 1) * NLOC].astype(ml_dtypes.float8_e4m3)
        in_maps.append({"qT": q8, "tT": np.ascontiguousarray(sh.T)})
    res = run_bass_kernel_spmd(nc, in_maps, core_ids=list(range(NCORES)),
                               trace=trace)
    if trace:
        _run_device.last_exec_ns = res.exec_time_ns

    # slot s of CPT=40 belongs to chunk g=s//8; device index p in [0,500)
    # is ambiguous over the group's GT=5 tiles -> expand to 5 twin
    # columns per candidate, each carrying the folded (max) value.
    g_of_slot = np.arange(CPT, dtype=np.int64) // 8          # [CPT]
    group_base = g_of_slot * (GT * TS)                        # [CPT]
    twin_off = np.arange(GT, dtype=np.int64) * TS             # [GT]
    cvs, cis = [], []
    for c in range(NCORES):
        v = res.results[c]["out_v"]                    # [NG, 128, BT*8]
        p = res.results[c]["out_i"]
        v = (v.reshape(NG, 128, BT, 8).transpose(2, 1, 0, 3)
             .reshape(B, CPT).astype(np.float32))
        p = (p.reshape(NG, 128, BT, 8).transpose(2, 1, 0, 3)
             .reshape(B, CPT).astype(np.int64))
        cols = (c * NLOC + group_base)[None, :, None] \
            + twin_off[None, None, :] + p[:, :, None]         # [B, CPT, GT]
        cvs.append(np.repeat(v, GT, axis=1))
        cis.append(cols.reshape(B, CPT * GT))
    return np.concatenate(cvs, axis=1), np.concatenate(cis, axis=1)


def kernel(features_rank, train_features, train_labels):
    q = np.ascontiguousarray(np.asarray(features_rank), dtype=np.float32)
    t = np.ascontiguousarray(np.asarray(train_features), dtype=np.float32)
    lab = np.asarray(train_labels)

    cv, ci = _run_device(q, t)

    part = np.argpartition(-cv, MAXK - 1, axis=1)[:, :MAXK]
    pv = np.take_along_axis(cv, part, axis=1)
    pi = np.take_along_axis(ci, part, axis=1)
    order = np.lexsort((pi, -pv), axis=1)
    topv = np.take_along_axis(pv, order, axis=1)
    topi = np.take_along_axis(pi, order, axis=1)

    # fp8 sims are approximate and twin values are inflated to the group
    # max: exactly rescore the top-RESCORE_POOL of every row (the true
    # top few are inside by a wide margin), re-sort, then softmax. The
    # tail keeps approx values; its true weights underflow to exactly 0
    # at T=0.07 regardless.
    p2 = RESCORE_POOL
    sub_i = topi[:, :p2]
    ex = np.empty((B, p2), np.float32)
    CH = 256
    for i in range(0, B, CH):
        ex[i:i + CH] = (t[sub_i[i:i + CH]] @ q[i:i + CH, :, None])[:, :, 0]
    o2 = np.lexsort((sub_i, -ex), axis=1)
    topv[:, :p2] = np.take_along_axis(ex, o2, axis=1)
    topi[:, :p2] = np.take_along_axis(sub_i, o2, axis=1)

    x = topv / np.float32(TEMP)
    x -= x.max(axis=1, keepdims=True)
    e = np.exp(x, dtype=np.float32)
    w = e / e.sum(axis=1, keepdims=True, dtype=np.float32)

    nl = lab[topi].astype(np.int64)
    flat_base = np.arange(B, dtype=np.int64)[:, None] * NUM_CLASSES
    probas = []
    for k in NB_KNN:
        p = np.bincount((nl[:, :k] + flat_base).ravel(),
                        weights=w[:, :k].astype(np.float64).ravel(),
                        minlength=B * NUM_CLASSES)
        probas.append(p.reshape(B, NUM_CLASSES).astype(np.float32))
    return tuple(probas)
